# revision 30
# baseline (speedup 1.0000x reference)
"""Trainium2 Bass kernel for nn_EntityEncoder (gnn_message_passing).

Contract: kernel(**inputs) takes the FULL unsharded inputs (numpy) and
returns the full outputs (context_entity_hidden [32,48,128],
kb_entity_hidden [32,512,128]) as a tuple, matching reference().

Strategy: data-parallel over the batch dim (4 batches per NeuronCore,
8 cores, one SPMD program). Gathers run on-device (indirect DMA /
onehot matmuls); the per-edge relation matvec uses a relation-sorted
32-wide slot layout with a per-core slot->weight table so the
instruction stream is identical on every core. kb_state rows are
stored sorted and un-sorted by an inverse-permutation gather feeding a
row-oriented PSUM-accumulated aggregation.
"""
import sys

sys.path.insert(0, "/opt/trn_rl_repo")

from contextlib import ExitStack

import numpy as np
import ml_dtypes

import concourse.bass as bass
import concourse.tile as tile
from concourse import bacc, mybir
from concourse.bass_utils import run_bass_kernel_spmd
from concourse.masks import make_identity

# problem shapes (hardcoded per spec)
B, L, EC, N, M, D, R, V = 32, 128, 48, 256, 512, 128, 100, 40000
NCORES = 8
BPC = B // NCORES          # batches per core = 4
SLOT = 32                  # edges per matvec slot (PE col-group width)
EDG = BPC * M              # edges per core = 2048
NE = BPC * N               # entities per core = 1024
KR = 16                    # slots per (batch, relation)
RPC = 8                    # relations per 512-col PSUM chunk
NCH = (R + RPC - 1) // RPC # chunks = 13
F32 = mybir.dt.float32
BF16 = mybir.dt.bfloat16
I32 = mybir.dt.int32
NPBF = ml_dtypes.bfloat16


# ---------------------------------------------------------------- host prep

def _host_prep(inputs):
    ce_emb = np.asarray(inputs["context_emb"], np.float32)
    ce_out = np.asarray(inputs["context_outputs"], np.float32)
    cmask = np.asarray(inputs["context_mask"], np.int32)
    cpos = np.asarray(inputs["context_entity_pos"], np.int32)
    cemask = np.asarray(inputs["context_entity_mask"], np.int32)
    entity = np.asarray(inputs["entity"], np.int32)
    kbe = np.asarray(inputs["kb_entity"], np.int32)
    kbm = np.asarray(inputs["kb_entity_mask"], np.int32)
    kbc = np.asarray(inputs["kb_entity_col"], np.int32)
    nei = np.asarray(inputs["kb_entity_nei"], np.int32)
    embed_table = np.asarray(inputs["embed_table"], np.float32)
    mlp1_w = np.asarray(inputs["mlp1_w"], np.float32)
    mlp1_b = np.asarray(inputs["mlp1_b"], np.float32)
    mlp2_w = np.asarray(inputs["mlp2_w"], np.float32)
    mlp2_b = np.asarray(inputs["mlp2_b"], np.float32)
    attn_wq = np.asarray(inputs["attn_wq"], np.float32)
    attn_bq = np.asarray(inputs["attn_bq"], np.float32)
    W = np.asarray(inputs["W"], np.float32)
    W0_w = np.asarray(inputs["W0_w"], np.float32)

    # fixed (relation, batch, slot16) grid: KR slots per (b, r); NCH chunks
    # of RPC=8 relations; all layout decisions static across cores.
    assert max(np.bincount(kbc[b], minlength=R).max() for b in range(B)) <= KR

    m1 = mlp1_w.T.reshape(2, 128, 128).transpose(1, 0, 2)
    m2 = mlp2_w.T.reshape(2, 128, 128).transpose(1, 0, 2)
    wpack = np.stack([m1[:, 0], m1[:, 1], m2[:, 0], m2[:, 1],
                      attn_wq.T, W0_w.T], axis=1)
    bpack = np.stack([mlp1_b, mlp2_b, attn_bq], axis=1)
    shared = dict(
        wpack=np.ascontiguousarray(wpack.astype(np.float32)),
        bpack=np.ascontiguousarray(bpack.astype(np.float32)),
        wtp=np.ascontiguousarray(W.transpose(2, 0, 1).astype(NPBF)),
        w0b=np.ascontiguousarray(W0_w.T.astype(NPBF)),
        emb_tbl=embed_table,
    )

    in_maps = []
    for c in range(NCORES):
        sl = slice(BPC * c, BPC * (c + 1))
        amask = np.ascontiguousarray(
            np.where(cmask[sl] > 0, 0.0, -1e9).astype(np.float32).reshape(
                1, BPC * 128))
        oh1 = np.zeros((BPC, L, EC), np.float32)
        for b in range(BPC):
            oh1[b, cpos[sl][b], np.arange(EC)] = cemask[sl][b].astype(np.float32)

        kbe_c, kbm_c = kbe[sl], kbm[sl]
        kbc_c = kbc[sl]
        # oho2: per-batch onehot into the (r, slot16) grid (no mask: the kb
        # mask is folded into nei, and pads never get read back)
        oh2 = np.zeros((BPC, N, R * KR), np.float32)
        inv = np.zeros(EDG, np.int32)
        for b in range(BPC):
            order = np.lexsort((np.arange(M), kbc_c[b]))
            i_in_r = np.zeros(R, np.int32)
            for m_ in order:
                r = kbc_c[b, m_]
                pos = r * KR + i_in_r[r]
                i_in_r[r] += 1
                oh2[b, kbe_c[b, m_], pos] = 1.0
                ch_, pc_ = pos // 128, pos % 128
                inv[b * M + m_] = ch_ * 512 + pc_ * 4 + b
        # chunked upload [NCH, 128, BPC, 2, RPC*KR] (zero-padded past R*KR)
        oh2p = np.zeros((BPC, N, NCH * RPC * KR), np.float32)
        oh2p[:, :, :R * KR] = oh2
        oh2c = np.ascontiguousarray(
            oh2p.reshape(BPC, 2, 128, NCH, RPC * KR)
                .transpose(3, 2, 0, 1, 4))  # [NCH, 128, BPC, 2, 128]

        ipack = np.ascontiguousarray(np.concatenate([
            entity[sl].ravel().reshape(NE // 128, 128).T.astype(np.int32),
            inv.reshape(EDG // 128, 128).T], axis=1))

        # original-order kb_init onehot (mask folded in) for the W0 term
        oho = np.zeros((BPC, N, M), np.float32)
        for b in range(BPC):
            oho[b, kbe_c[b], np.arange(M)] = kbm_c[b].astype(np.float32)
        oh_orig = np.ascontiguousarray(
            oho.reshape(BPC, 2, 128, M).transpose(0, 2, 1, 3).astype(NPBF))

        # degree-normalized transposed neighbor matrix (mask folded in)
        nei_c = nei[sl].astype(np.float32)
        deg = np.clip(nei_c.sum(axis=2), 1.0, None)
        nnT = (nei_c / deg[:, :, None]).transpose(0, 2, 1)
        nnT = nnT * kbm_c.astype(np.float32)[:, :, None]
        nei_t = np.ascontiguousarray(
            nnT.reshape(BPC, 4, 128, M).transpose(0, 2, 1, 3).reshape(
                BPC, 128, 4 * M).astype(NPBF))

        m = dict(shared)
        m.update(
            ce_emb=np.ascontiguousarray(ce_emb[sl]),
            ce_out=np.ascontiguousarray(ce_out[sl]),
            amask=amask,
            onehot1=np.ascontiguousarray(oh1.transpose(1, 0, 2)),
            ipack=ipack,
            oh2c=oh2c.astype(NPBF),
            oh_orig=oh_orig,
            nei_t=nei_t,
        )
        in_maps.append(m)
    return in_maps


# ------------------------------------------------------------- bass program

def _build_program():
    nto = EDG // 128
    nc = bacc.Bacc("TRN2", target_bir_lowering=False, debug=False,
                   num_devices=NCORES)

    def din(name, shape, dt=F32):
        return nc.dram_tensor(name, list(shape), dt, kind="ExternalInput").ap()

    ce_emb = din("ce_emb", (BPC, 128, 128))
    ce_out = din("ce_out", (BPC, 128, 128))
    amask = din("amask", (1, BPC * 128))
    onehot1 = din("onehot1", (128, BPC, EC))
    wpack = din("wpack", (128, 6, 128))
    bpack = din("bpack", (128, 3))
    wtp = din("wtp", (128, R, 128), BF16)
    w0b_d = din("w0b", (128, 128), BF16)
    ipack = din("ipack", (128, NE // 128 + nto), I32)
    emb_tbl = din("emb_tbl", (V, 128))
    oh2c_d = din("oh2c", (NCH, 128, BPC, 2, RPC * KR), BF16)
    oh_orig = din("oh_orig", (BPC, 128, 2, M), BF16)
    nei_t = din("nei_t", (BPC, 128, 4 * M), BF16)

    out_ctx = nc.dram_tensor("out_ctx", [BPC * EC, 128], F32,
                             kind="ExternalOutput").ap()
    out_kb = nc.dram_tensor("out_kb", [EDG, 128], F32,
                            kind="ExternalOutput").ap()

    ks_sort = nc.dram_tensor("ks_sort", [(NCH - 1) * 512 + 256, 128], BF16).ap()

    with tile.TileContext(nc) as tc, ExitStack() as ctx:
        consts = ctx.enter_context(tc.tile_pool(name="consts", bufs=1))
        big = ctx.enter_context(tc.tile_pool(name="big", bufs=1))
        work = ctx.enter_context(tc.tile_pool(name="work", bufs=3))
        work2 = ctx.enter_context(tc.tile_pool(name="work2", bufs=3))
        keep = ctx.enter_context(tc.tile_pool(name="keep", bufs=1))
        small = ctx.enter_context(tc.tile_pool(name="small", bufs=4))

        # ---- gpsimd: identity + ones row first
        ident = consts.tile([128, 128], F32)
        make_identity(nc, ident[:])
        ones1 = consts.tile([1, 128], F32)
        nc.gpsimd.memset(ones1[:], 1.0)
        identb = consts.tile([128, 128], BF16)
        nc.vector.tensor_copy(out=identb[:], in_=ident[:])

        # ---- sync queue: small latency-critical loads, indices first
        def ld(pool, shape, src, dt=F32, name=None):
            t = pool.tile(shape, dt, name=name)
            nc.sync.dma_start(out=t[:], in_=src)
            return t

        ipk = ld(consts, [128, NE // 128 + nto], ipack[:], I32, "ipk")
        eidx = ipk[:, 0:NE // 128]
        vidx = ipk[:, NE // 128:]
        oh1 = ld(consts, [128, BPC, EC], onehot1[:], name="oh1")
        cem = [ld(consts, [128, 128], ce_emb[b], name=f"cem{b}")
               for b in range(BPC)]
        ceo = [ld(consts, [128, 128], ce_out[b], name=f"ceo{b}")
               for b in range(BPC)]
        wpk = ld(consts, [128, 6, 128], wpack[:], name="wpk")
        bpk = ld(consts, [128, 3], bpack[:], name="bpk")
        amr = ld(consts, [1, BPC * 128], amask[:], name="amr")
        w0b = ld(consts, [128, 128], w0b_d[:], BF16, "w0b")

        # ---- gpsimd: embedding gathers early (phase B input)
        embr = [[keep.tile([128, 128], F32, name=f"embr{b}_{j}")
                 for j in range(2)] for b in range(BPC)]
        for b in range(BPC):
            for j in range(2):
                nc.gpsimd.indirect_dma_start(
                    out=embr[b][j][:], out_offset=None, in_=emb_tbl[:],
                    in_offset=bass.IndirectOffsetOnAxis(
                        ap=eidx[:, 2 * b + j:2 * b + j + 1], axis=0))

        # ---- bulk background loads on sync HWDGE rings
        oho_sb = [big.tile([128, 2, M], BF16, name=f"oho{b}") for b in range(BPC)]
        for b in range(BPC):
            nc.sync.dma_start(out=oho_sb[b][:], in_=oh_orig[b])
        wt_sb = big.tile([128, R, 128], BF16)
        for q in range(4):
            qs = R // 4
            nc.sync.dma_start(out=wt_sb[:, q * qs:(q + 1) * qs, :],
                              in_=wtp[:, q * qs:(q + 1) * qs, :])
        nei_sb = [big.tile([128, 4 * M], BF16, name=f"nei{b}") for b in range(BPC)]

        m1b, m2b, bqs = bpk[:, 0:1], bpk[:, 1:2], bpk[:, 2:3]

        with tc.tile_pool(name="psA", bufs=2, space="PSUM") as psA, \
             tc.tile_pool(name="psB", bufs=5, space="PSUM") as psB:

            # ================= phase A: context-entity hidden ================
            cehT = [keep.tile([128, BPC * EC], F32, name=f"cehT{k}")
                    for k in range(2)]
            for b in range(BPC):
                for k, src in ((0, cem[b]), (1, ceo[b])):
                    aps = psA.tile([128, EC], F32, space="PSUM", tag="a")
                    nc.tensor.matmul(out=aps[:], lhsT=src[:],
                                     rhs=oh1[:, b, :], start=True, stop=True)
                    nc.scalar.copy(out=cehT[k][:, b * EC:(b + 1) * EC],
                                   in_=aps[:])
            o1ps = psB.tile([128, BPC * EC], F32, space="PSUM", tag="b")
            nc.tensor.matmul(out=o1ps[:], lhsT=wpk[:, 0, :], rhs=cehT[0][:],
                             start=True, stop=False)
            nc.tensor.matmul(out=o1ps[:], lhsT=wpk[:, 1, :], rhs=cehT[1][:],
                             start=False, stop=True)
            o1T = work.tile([128, BPC * EC], F32)
            nc.scalar.activation(out=o1T[:], in_=o1ps[:],
                                 func=mybir.ActivationFunctionType.Relu,
                                 bias=m1b)
            for h in range(2):
                tp = psA.tile([96, 128], F32, space="PSUM", tag="a")
                nc.tensor.transpose(out=tp[:], in_=o1T[:, h * 96:(h + 1) * 96],
                                    identity=ident[:])
                o1r = work.tile([96, 128], F32)
                nc.vector.tensor_copy(out=o1r[:], in_=tp[:])
                nc.sync.dma_start(out=out_ctx[h * 96:(h + 1) * 96, :],
                                  in_=o1r[:])

            # ================= phase B: entity attention + mlp2 ==============
            ceT = [keep.tile([128, 128], F32, name=f"ceT{b}")
                   for b in range(BPC)]
            embT = keep.tile([128, BPC * N], F32)
            for b in range(BPC):
                tp = psA.tile([128, 128], F32, space="PSUM", tag="a")
                nc.tensor.transpose(out=tp[:], in_=cem[b][:], identity=ident[:])
                nc.scalar.copy(out=ceT[b][:], in_=tp[:])
                for j in range(2):
                    tp2 = psA.tile([128, 128], F32, space="PSUM", tag="a")
                    nc.tensor.transpose(out=tp2[:], in_=embr[b][j][:],
                                        identity=ident[:])
                    nc.vector.tensor_copy(
                        out=embT[:, b * N + j * 128:b * N + (j + 1) * 128],
                        in_=tp2[:])

            qT = keep.tile([128, BPC * N], F32)
            for h in range(2):
                qps = psB.tile([128, 512], F32, space="PSUM", tag="b")
                nc.tensor.matmul(out=qps[:], lhsT=wpk[:, 4, :],
                                 rhs=embT[:, h * 512:(h + 1) * 512],
                                 start=True, stop=True)
                nc.scalar.activation(out=qT[:, h * 512:(h + 1) * 512],
                                     in_=qps[:],
                                     func=mybir.ActivationFunctionType.Identity,
                                     bias=bqs)

            sps = []
            for b in range(BPC):
                ps = psB.tile([128, 256], F32, space="PSUM", tag="b")
                sps.append(ps)
                for ntl in range(2):
                    nc.tensor.matmul(
                        out=ps[:, ntl * 128:(ntl + 1) * 128],
                        lhsT=qT[:, b * N + ntl * 128:b * N + (ntl + 1) * 128],
                        rhs=ceT[b][:], start=(ntl == 0), stop=False)
                for ntl in range(2):
                    nc.tensor.matmul(
                        out=ps[:, ntl * 128:(ntl + 1) * 128],
                        lhsT=ones1[:], rhs=amr[:, b * 128:(b + 1) * 128],
                        start=False, stop=(ntl == 1))

            att = []
            for b in range(BPC):
                sc = small.tile([128, 2, 128], F32, name="sc")
                nc.scalar.activation(out=sc[:], in_=sps[b][:].rearrange(
                    "p (t l) -> p t l", t=2),
                    func=mybir.ActivationFunctionType.Exp)
                ssum = small.tile([128, 2], F32, name="ssum")
                nc.vector.tensor_reduce(out=ssum[:], in_=sc[:],
                                        axis=mybir.AxisListType.X,
                                        op=mybir.AluOpType.add)
                rs = small.tile([128, 2], F32, name="rs")
                nc.vector.reciprocal(out=rs[:], in_=ssum[:])
                rsb = bass.AP(tensor=rs.tensor, offset=rs.offset,
                              ap=[rs.ap[0], [rs.ap[1][0], 2], [0, 128]])
                nc.vector.tensor_tensor(out=sc[:], in0=sc[:], in1=rsb,
                                        op=mybir.AluOpType.mult)
                att.append(sc)

            alT = keep.tile([128, BPC * N], F32)
            for b in range(BPC):
                awT = work.tile([128, N], F32)
                for ntl in range(2):
                    tp3 = psA.tile([128, 128], F32, space="PSUM", tag="a")
                    nc.tensor.transpose(out=tp3[:], in_=att[b][:, ntl, :],
                                        identity=ident[:])
                    nc.vector.tensor_copy(out=awT[:, ntl * 128:(ntl + 1) * 128],
                                          in_=tp3[:])
                alps = psB.tile([128, N], F32, space="PSUM", tag="b")
                nc.tensor.matmul(out=alps[:], lhsT=cem[b][:], rhs=awT[:],
                                 start=True, stop=True)
                nc.scalar.copy(out=alT[:, b * N:(b + 1) * N], in_=alps[:])

            ehr = [[keep.tile([128, 128], BF16, name=f"ehr{b}_{j}")
                    for j in range(2)] for b in range(BPC)]
            for h in range(2):
                ehps = psB.tile([128, 512], F32, space="PSUM", tag="b")
                nc.tensor.matmul(out=ehps[:], lhsT=wpk[:, 2, :],
                                 rhs=embT[:, h * 512:(h + 1) * 512],
                                 start=True, stop=False)
                nc.tensor.matmul(out=ehps[:], lhsT=wpk[:, 3, :],
                                 rhs=alT[:, h * 512:(h + 1) * 512],
                                 start=False, stop=True)
                ehT = work.tile([128, 512], F32, name="ehT")
                nc.scalar.activation(out=ehT[:], in_=ehps[:],
                                     func=mybir.ActivationFunctionType.Relu,
                                     bias=m2b)
                for j in range(4):
                    b = (h * 512 + j * 128) // N
                    jj = ((h * 512 + j * 128) % N) // 128
                    tp4 = psA.tile([128, 128], F32, space="PSUM", tag="a")
                    nc.tensor.transpose(out=tp4[:],
                                        in_=ehT[:, j * 128:(j + 1) * 128],
                                        identity=ident[:])
                    nc.vector.tensor_copy(out=ehr[b][jj][:], in_=tp4[:])

        # ================= phase C: KB graph ============================
        kbiT = [keep.tile([128, M], BF16, name=f"kbiT{b}") for b in range(BPC)]

        with tc.tile_pool(name="psS", bufs=3, space="PSUM") as psS, \
             tc.tile_pool(name="psV", bufs=3, space="PSUM") as psV, \
             tc.tile_pool(name="psT", bufs=2, space="PSUM") as psT:

            # original-order kb_init via onehot matmul (mask folded in)
            for b in range(BPC):
                kps = psV.tile([128, M], F32, space="PSUM", tag="v")
                for kt in range(2):
                    nc.tensor.matmul(out=kps[:], lhsT=ehr[b][kt][:],
                                     rhs=oho_sb[b][:, kt, :],
                                     start=(kt == 0), stop=(kt == 1))
                nc.scalar.copy(out=kbiT[b][:], in_=kps[:])

            # chunked pipeline; psum layouts stay 2D, the two PSUM->SBUF
            # copies do the (b, r, i) <-> (r, b, i) reshuffles via 4D APs
            CW = RPC * KR * BPC  # 512 psum cols per chunk
            for ch in range(NCH):
                rlo = ch * RPC
                nrel = min(RPC, R - rlo)
                oh2t = work2.tile([128, BPC, 2, RPC * KR], BF16, name="oh2t")
                nc.sync.dma_start(out=oh2t[:], in_=oh2c_d[ch])
                # build kb_init chunk in (b, r, i) layout: 2D outs
                bps = psS.tile([128, CW], F32, space="PSUM", tag="s")
                for b in range(BPC):
                    for kt in range(2):
                        nc.tensor.matmul(
                            out=bps[:, b * 128:(b + 1) * 128],
                            lhsT=ehr[b][kt][:],
                            rhs=oh2t[:, b, kt, :],
                            start=(b == 0 and kt == 0),
                            stop=(b == BPC - 1 and kt == 1))
                # copy + reshuffle (b, r, i) -> (r, b, i)
                kbc_t = work2.tile([128, CW], BF16, name="kbc_t")
                kbc_w = kbc_t[:].rearrange("p (r b i) -> p b r i",
                                           r=RPC, b=BPC, i=KR)
                bps_r = bps[:].rearrange("p (b r i) -> p b r i",
                                         b=BPC, r=RPC, i=KR)
                nc.vector.tensor_copy(out=kbc_w, in_=bps_r)
                # per-relation matvec: contiguous (r, b, i) slices
                vps = psV.tile([128, CW], F32, space="PSUM", tag="v")
                for j in range(nrel):
                    nc.tensor.matmul(
                        out=vps[:, j * BPC * KR:(j + 1) * BPC * KR],
                        lhsT=wt_sb[:, rlo + j, :],
                        rhs=kbc_t[:, j * BPC * KR:(j + 1) * BPC * KR],
                        start=(j == 0), stop=(j == nrel - 1))
                # copy + reshuffle back (r, b, i) -> (b, r, i); only the
                # nrel*BPC*KR cols actually written (last chunk is partial)
                nw = nrel * KR
                ksc = work2.tile([128, CW], BF16, name="ksc")
                ksc_w = ksc[:].rearrange("p (b r i) -> p b r i",
                                         b=BPC, r=RPC, i=KR)[:, :, :nrel, :]
                vps_r = vps[:, :nrel * BPC * KR].rearrange(
                    "p (r b i) -> p b r i", r=nrel, b=BPC, i=KR)
                nc.scalar.copy(out=ksc_w, in_=vps_r)
                # per-batch transposes -> sorted kb_state rows, one store
                kso = work2.tile([128, BPC, 128], BF16, name="kso")
                for b in range(BPC):
                    tps = psT.tile([128, 128], BF16, space="PSUM", tag="t")
                    nc.tensor.transpose(out=tps[:nw, :],
                                        in_=ksc[:, b * 128:b * 128 + nw],
                                        identity=identb[:])
                    nc.vector.tensor_copy(out=kso[:nw, b, :], in_=tps[:nw, :])
                base = ch * 512
                nc.sync.dma_start(
                    out=ks_sort[base:base + nw * BPC, :].rearrange(
                        "(p b) d -> p b d", b=BPC),
                    in_=kso[:nw, :, :])

            for b in range(BPC):
                nc.sync.dma_start(out=nei_sb[b][:], in_=nei_t[b])

        # aggregation (row-oriented): out rows = relu(
        #   kbi_T-slice.T @ W0_w.T + sum_kt nei_T-slice.T @ ks_rows[kt])
        with tc.tile_pool(name="psH", bufs=4, space="PSUM") as psH:
            for b in range(BPC):
                ksb = [small.tile([128, 128], BF16, name="ksb", bufs=8)
                       for _ in range(4)]
                for kt in range(4):
                    nc.gpsimd.indirect_dma_start(
                        out=ksb[kt][:], out_offset=None, in_=ks_sort[:],
                        in_offset=bass.IndirectOffsetOnAxis(
                            ap=vidx[:, 4 * b + kt:4 * b + kt + 1], axis=0))
                for mt in range(4):
                    hps = psH.tile([128, 128], F32, space="PSUM", tag="h")
                    nc.tensor.matmul(
                        out=hps[:], lhsT=kbiT[b][:, mt * 128:(mt + 1) * 128],
                        rhs=w0b[:], start=True, stop=False)
                    for kt in range(4):
                        nc.tensor.matmul(
                            out=hps[:],
                            lhsT=nei_sb[b][:, kt * M + mt * 128:
                                           kt * M + (mt + 1) * 128],
                            rhs=ksb[kt][:], start=False, stop=(kt == 3))
                    orow = small.tile([128, 128], F32, name="orow")
                    nc.scalar.activation(
                        out=orow[:], in_=hps[:],
                        func=mybir.ActivationFunctionType.Relu)
                    nc.sync.dma_start(
                        out=out_kb[b * M + mt * 128:b * M + (mt + 1) * 128, :],
                        in_=orow[:])

    nc.compile()
    return nc


_CACHE = {}


def _get_program():
    if "nc" not in _CACHE:
        _CACHE["nc"] = _build_program()
    return _CACHE["nc"]


def kernel(**inputs):
    in_maps = _host_prep(inputs)
    nc = _get_program()
    res = run_bass_kernel_spmd(nc, in_maps, list(range(NCORES)))
    out_ctx = np.concatenate(
        [res.results[c]["out_ctx"].reshape(BPC, EC, 128) for c in range(NCORES)])
    out_kb = np.concatenate(
        [res.results[c]["out_kb"].reshape(BPC, M, 128) for c in range(NCORES)])
    return out_ctx, out_kb


# revision 32
# speedup vs baseline: 1.0297x; 1.0297x over previous
"""Trainium2 Bass kernel for nn_EntityEncoder (gnn_message_passing).

Contract: kernel(**inputs) takes the FULL unsharded inputs (numpy) and
returns the full outputs (context_entity_hidden [32,48,128],
kb_entity_hidden [32,512,128]) as a tuple, matching reference().

Strategy: data-parallel over the batch dim (4 batches per NeuronCore,
8 cores, one SPMD program). Gathers run on-device (indirect DMA /
onehot matmuls); the per-edge relation matvec uses a relation-sorted
32-wide slot layout with a per-core slot->weight table so the
instruction stream is identical on every core. kb_state rows are
stored sorted and un-sorted by an inverse-permutation gather feeding a
row-oriented PSUM-accumulated aggregation.
"""
import sys

sys.path.insert(0, "/opt/trn_rl_repo")

from contextlib import ExitStack

import numpy as np
import ml_dtypes

import concourse.bass as bass
import concourse.tile as tile
from concourse import bacc, mybir
from concourse.bass_utils import run_bass_kernel_spmd
from concourse.masks import make_identity

# problem shapes (hardcoded per spec)
B, L, EC, N, M, D, R, V = 32, 128, 48, 256, 512, 128, 100, 40000
NCORES = 8
BPC = B // NCORES          # batches per core = 4
SLOT = 32                  # edges per matvec slot (PE col-group width)
EDG = BPC * M              # edges per core = 2048
NE = BPC * N               # entities per core = 1024
KR = 16                    # slots per (batch, relation)
RPC = 8                    # relations per 512-col PSUM chunk
NCH = (R + RPC - 1) // RPC # chunks = 13
F32 = mybir.dt.float32
BF16 = mybir.dt.bfloat16
I32 = mybir.dt.int32
NPBF = ml_dtypes.bfloat16


# ---------------------------------------------------------------- host prep

def _host_prep(inputs):
    ce_emb = np.asarray(inputs["context_emb"], np.float32)
    ce_out = np.asarray(inputs["context_outputs"], np.float32)
    cmask = np.asarray(inputs["context_mask"], np.int32)
    cpos = np.asarray(inputs["context_entity_pos"], np.int32)
    cemask = np.asarray(inputs["context_entity_mask"], np.int32)
    entity = np.asarray(inputs["entity"], np.int32)
    kbe = np.asarray(inputs["kb_entity"], np.int32)
    kbm = np.asarray(inputs["kb_entity_mask"], np.int32)
    kbc = np.asarray(inputs["kb_entity_col"], np.int32)
    nei = np.asarray(inputs["kb_entity_nei"], np.int32)
    embed_table = np.asarray(inputs["embed_table"], np.float32)
    mlp1_w = np.asarray(inputs["mlp1_w"], np.float32)
    mlp1_b = np.asarray(inputs["mlp1_b"], np.float32)
    mlp2_w = np.asarray(inputs["mlp2_w"], np.float32)
    mlp2_b = np.asarray(inputs["mlp2_b"], np.float32)
    attn_wq = np.asarray(inputs["attn_wq"], np.float32)
    attn_bq = np.asarray(inputs["attn_bq"], np.float32)
    W = np.asarray(inputs["W"], np.float32)
    W0_w = np.asarray(inputs["W0_w"], np.float32)

    # fixed (relation, batch, slot16) grid: KR slots per (b, r); NCH chunks
    # of RPC=8 relations; all layout decisions static across cores.
    assert max(np.bincount(kbc[b], minlength=R).max() for b in range(B)) <= KR

    m1 = mlp1_w.T.reshape(2, 128, 128).transpose(1, 0, 2)
    m2 = mlp2_w.T.reshape(2, 128, 128).transpose(1, 0, 2)
    wpack = np.stack([m1[:, 0], m1[:, 1], m2[:, 0], m2[:, 1],
                      attn_wq.T, W0_w.T], axis=1)
    bpack = np.stack([mlp1_b, mlp2_b, attn_bq], axis=1)
    shared = dict(
        wpack=np.ascontiguousarray(wpack.astype(np.float32)),
        bpack=np.ascontiguousarray(bpack.astype(np.float32)),
        wtp=np.ascontiguousarray(W.transpose(2, 0, 1).astype(NPBF)),
        w0b=np.ascontiguousarray(W0_w.T.astype(NPBF)),
        emb_tbl=embed_table,
    )

    in_maps = []
    for c in range(NCORES):
        sl = slice(BPC * c, BPC * (c + 1))
        amask = np.ascontiguousarray(
            np.where(cmask[sl] > 0, 0.0, -1e9).astype(np.float32).reshape(
                1, BPC * 128))
        oh1 = np.zeros((BPC, L, EC), np.float32)
        for b in range(BPC):
            oh1[b, cpos[sl][b], np.arange(EC)] = cemask[sl][b].astype(np.float32)

        kbe_c, kbm_c = kbe[sl], kbm[sl]
        kbc_c = kbc[sl]
        # oho2: per-batch onehot into the (r, slot16) grid (no mask: the kb
        # mask is folded into nei, and pads never get read back)
        oh2 = np.zeros((BPC, N, R * KR), np.float32)
        inv = np.zeros(EDG, np.int32)
        for b in range(BPC):
            order = np.lexsort((np.arange(M), kbc_c[b]))
            i_in_r = np.zeros(R, np.int32)
            for m_ in order:
                r = kbc_c[b, m_]
                pos = r * KR + i_in_r[r]
                i_in_r[r] += 1
                oh2[b, kbe_c[b, m_], pos] = 1.0
                ch_, pc_ = pos // 128, pos % 128
                inv[b * M + m_] = ch_ * 512 + pc_ * 4 + b
        # chunked upload [NCH, 128, BPC, 2, RPC*KR] (zero-padded past R*KR)
        oh2p = np.zeros((BPC, N, NCH * RPC * KR), np.float32)
        oh2p[:, :, :R * KR] = oh2
        oh2c = np.ascontiguousarray(
            oh2p.reshape(BPC, 2, 128, NCH, RPC * KR)
                .transpose(3, 2, 0, 1, 4))  # [NCH, 128, BPC, 2, 128]

        ipack = np.ascontiguousarray(np.concatenate([
            entity[sl].ravel().reshape(NE // 128, 128).T.astype(np.int32),
            inv.reshape(EDG // 128, 128).T], axis=1))

        # original-order kb_init onehot (mask folded in) for the W0 term
        oho = np.zeros((BPC, N, M), np.float32)
        for b in range(BPC):
            oho[b, kbe_c[b], np.arange(M)] = kbm_c[b].astype(np.float32)
        oh_orig = np.ascontiguousarray(
            oho.reshape(BPC, 2, 128, M).transpose(0, 2, 1, 3).astype(NPBF))

        # degree-normalized transposed neighbor matrix (mask folded in)
        nei_c = nei[sl].astype(np.float32)
        deg = np.clip(nei_c.sum(axis=2), 1.0, None)
        nnT = (nei_c / deg[:, :, None]).transpose(0, 2, 1)
        nnT = nnT * kbm_c.astype(np.float32)[:, :, None]
        nei_t = np.ascontiguousarray(
            nnT.reshape(BPC, 4, 128, M).transpose(0, 2, 1, 3).reshape(
                BPC, 128, 4 * M).astype(NPBF))

        m = dict(shared)
        m.update(
            ce_emb=np.ascontiguousarray(ce_emb[sl]),
            ce_out=np.ascontiguousarray(ce_out[sl]),
            amask=amask,
            onehot1=np.ascontiguousarray(oh1.transpose(1, 0, 2)),
            ipack=ipack,
            oh2c=oh2c.astype(NPBF),
            oh_orig=oh_orig,
            nei_t=nei_t,
        )
        in_maps.append(m)
    return in_maps


# ------------------------------------------------------------- bass program

def _build_program():
    nto = EDG // 128
    nc = bacc.Bacc("TRN2", target_bir_lowering=False, debug=False,
                   num_devices=NCORES)

    def din(name, shape, dt=F32):
        return nc.dram_tensor(name, list(shape), dt, kind="ExternalInput").ap()

    ce_emb = din("ce_emb", (BPC, 128, 128))
    ce_out = din("ce_out", (BPC, 128, 128))
    amask = din("amask", (1, BPC * 128))
    onehot1 = din("onehot1", (128, BPC, EC))
    wpack = din("wpack", (128, 6, 128))
    bpack = din("bpack", (128, 3))
    wtp = din("wtp", (128, R, 128), BF16)
    w0b_d = din("w0b", (128, 128), BF16)
    ipack = din("ipack", (128, NE // 128 + nto), I32)
    emb_tbl = din("emb_tbl", (V, 128))
    oh2c_d = din("oh2c", (NCH, 128, BPC, 2, RPC * KR), BF16)
    oh_orig = din("oh_orig", (BPC, 128, 2, M), BF16)
    nei_t = din("nei_t", (BPC, 128, 4 * M), BF16)

    out_ctx = nc.dram_tensor("out_ctx", [BPC * EC, 128], F32,
                             kind="ExternalOutput").ap()
    out_kb = nc.dram_tensor("out_kb", [EDG, 128], F32,
                            kind="ExternalOutput").ap()

    ks_sort = nc.dram_tensor("ks_sort", [(NCH - 1) * 512 + 256, 128], BF16).ap()

    with tile.TileContext(nc) as tc, ExitStack() as ctx:
        consts = ctx.enter_context(tc.tile_pool(name="consts", bufs=1))
        big = ctx.enter_context(tc.tile_pool(name="big", bufs=1))
        work = ctx.enter_context(tc.tile_pool(name="work", bufs=4))
        work2 = ctx.enter_context(tc.tile_pool(name="work2", bufs=3))
        keep = ctx.enter_context(tc.tile_pool(name="keep", bufs=1))
        small = ctx.enter_context(tc.tile_pool(name="small", bufs=6))

        # ---- gpsimd: identity + ones row first
        ident = consts.tile([128, 128], F32)
        make_identity(nc, ident[:])
        ones1 = consts.tile([1, 128], F32)
        nc.gpsimd.memset(ones1[:], 1.0)
        identb = consts.tile([128, 128], BF16)
        nc.vector.tensor_copy(out=identb[:], in_=ident[:])

        # ---- sync queue: small latency-critical loads, indices first
        def ld(pool, shape, src, dt=F32, name=None):
            t = pool.tile(shape, dt, name=name)
            nc.sync.dma_start(out=t[:], in_=src)
            return t

        ipk = ld(consts, [128, NE // 128 + nto], ipack[:], I32, "ipk")
        eidx = ipk[:, 0:NE // 128]
        vidx = ipk[:, NE // 128:]
        oh1 = ld(consts, [128, BPC, EC], onehot1[:], name="oh1")
        cem = [ld(consts, [128, 128], ce_emb[b], name=f"cem{b}")
               for b in range(BPC)]
        ceo = [ld(consts, [128, 128], ce_out[b], name=f"ceo{b}")
               for b in range(BPC)]
        wpk = ld(consts, [128, 6, 128], wpack[:], name="wpk")
        bpk = ld(consts, [128, 3], bpack[:], name="bpk")
        amr = ld(consts, [1, BPC * 128], amask[:], name="amr")
        w0b = ld(consts, [128, 128], w0b_d[:], BF16, "w0b")

        # ---- gpsimd: embedding gathers early (phase B input)
        embr = [[keep.tile([128, 128], F32, name=f"embr{b}_{j}")
                 for j in range(2)] for b in range(BPC)]
        for b in range(BPC):
            for j in range(2):
                nc.gpsimd.indirect_dma_start(
                    out=embr[b][j][:], out_offset=None, in_=emb_tbl[:],
                    in_offset=bass.IndirectOffsetOnAxis(
                        ap=eidx[:, 2 * b + j:2 * b + j + 1], axis=0))

        # ---- bulk background loads on sync HWDGE rings
        oho_sb = [big.tile([128, 2, M], BF16, name=f"oho{b}") for b in range(BPC)]
        for b in range(BPC):
            nc.sync.dma_start(out=oho_sb[b][:], in_=oh_orig[b])
        wt_sb = big.tile([128, R, 128], BF16)
        for q in range(4):
            qs = R // 4
            nc.sync.dma_start(out=wt_sb[:, q * qs:(q + 1) * qs, :],
                              in_=wtp[:, q * qs:(q + 1) * qs, :])
        nei_sb = [big.tile([128, 4 * M], BF16, name=f"nei{b}") for b in range(BPC)]

        m1b, m2b, bqs = bpk[:, 0:1], bpk[:, 1:2], bpk[:, 2:3]

        with tc.tile_pool(name="psA", bufs=2, space="PSUM") as psA, \
             tc.tile_pool(name="psB", bufs=5, space="PSUM") as psB:

            # ================= phase A: context-entity hidden ================
            cehT = [keep.tile([128, BPC * EC], F32, name=f"cehT{k}")
                    for k in range(2)]
            for b in range(BPC):
                for k, src in ((0, cem[b]), (1, ceo[b])):
                    aps = psA.tile([128, EC], F32, space="PSUM", tag="a")
                    nc.tensor.matmul(out=aps[:], lhsT=src[:],
                                     rhs=oh1[:, b, :], start=True, stop=True)
                    nc.scalar.copy(out=cehT[k][:, b * EC:(b + 1) * EC],
                                   in_=aps[:])
            o1ps = psB.tile([128, BPC * EC], F32, space="PSUM", tag="b")
            nc.tensor.matmul(out=o1ps[:], lhsT=wpk[:, 0, :], rhs=cehT[0][:],
                             start=True, stop=False)
            nc.tensor.matmul(out=o1ps[:], lhsT=wpk[:, 1, :], rhs=cehT[1][:],
                             start=False, stop=True)
            o1T = work.tile([128, BPC * EC], F32)
            nc.scalar.activation(out=o1T[:], in_=o1ps[:],
                                 func=mybir.ActivationFunctionType.Relu,
                                 bias=m1b)
            for h in range(2):
                tp = psA.tile([96, 128], F32, space="PSUM", tag="a")
                nc.tensor.transpose(out=tp[:], in_=o1T[:, h * 96:(h + 1) * 96],
                                    identity=ident[:])
                o1r = work.tile([96, 128], F32)
                nc.vector.tensor_copy(out=o1r[:], in_=tp[:])
                nc.sync.dma_start(out=out_ctx[h * 96:(h + 1) * 96, :],
                                  in_=o1r[:])

            # ================= phase B: entity attention + mlp2 ==============
            ceT = [keep.tile([128, 128], F32, name=f"ceT{b}")
                   for b in range(BPC)]
            embT = keep.tile([128, BPC * N], F32)
            for b in range(BPC):
                tp = psA.tile([128, 128], F32, space="PSUM", tag="a")
                nc.tensor.transpose(out=tp[:], in_=cem[b][:], identity=ident[:])
                nc.scalar.copy(out=ceT[b][:], in_=tp[:])
                for j in range(2):
                    tp2 = psA.tile([128, 128], F32, space="PSUM", tag="a")
                    nc.tensor.transpose(out=tp2[:], in_=embr[b][j][:],
                                        identity=ident[:])
                    nc.vector.tensor_copy(
                        out=embT[:, b * N + j * 128:b * N + (j + 1) * 128],
                        in_=tp2[:])

            qT = keep.tile([128, BPC * N], F32)
            for h in range(2):
                qps = psB.tile([128, 512], F32, space="PSUM", tag="b")
                nc.tensor.matmul(out=qps[:], lhsT=wpk[:, 4, :],
                                 rhs=embT[:, h * 512:(h + 1) * 512],
                                 start=True, stop=True)
                nc.scalar.activation(out=qT[:, h * 512:(h + 1) * 512],
                                     in_=qps[:],
                                     func=mybir.ActivationFunctionType.Identity,
                                     bias=bqs)

            sps = []
            for b in range(BPC):
                ps = psB.tile([128, 256], F32, space="PSUM", tag="b")
                sps.append(ps)
                for ntl in range(2):
                    nc.tensor.matmul(
                        out=ps[:, ntl * 128:(ntl + 1) * 128],
                        lhsT=qT[:, b * N + ntl * 128:b * N + (ntl + 1) * 128],
                        rhs=ceT[b][:], start=(ntl == 0), stop=False)
                for ntl in range(2):
                    nc.tensor.matmul(
                        out=ps[:, ntl * 128:(ntl + 1) * 128],
                        lhsT=ones1[:], rhs=amr[:, b * 128:(b + 1) * 128],
                        start=False, stop=(ntl == 1))

            att = []
            for b in range(BPC):
                sc = small.tile([128, 2, 128], F32, name="sc")
                nc.scalar.activation(out=sc[:], in_=sps[b][:].rearrange(
                    "p (t l) -> p t l", t=2),
                    func=mybir.ActivationFunctionType.Exp)
                ssum = small.tile([128, 2], F32, name="ssum")
                nc.vector.tensor_reduce(out=ssum[:], in_=sc[:],
                                        axis=mybir.AxisListType.X,
                                        op=mybir.AluOpType.add)
                rs = small.tile([128, 2], F32, name="rs")
                nc.vector.reciprocal(out=rs[:], in_=ssum[:])
                rsb = bass.AP(tensor=rs.tensor, offset=rs.offset,
                              ap=[rs.ap[0], [rs.ap[1][0], 2], [0, 128]])
                nc.vector.tensor_tensor(out=sc[:], in0=sc[:], in1=rsb,
                                        op=mybir.AluOpType.mult)
                att.append(sc)

            alT = keep.tile([128, BPC * N], F32)
            for b in range(BPC):
                awT = work.tile([128, N], F32)
                for ntl in range(2):
                    tp3 = psA.tile([128, 128], F32, space="PSUM", tag="a")
                    nc.tensor.transpose(out=tp3[:], in_=att[b][:, ntl, :],
                                        identity=ident[:])
                    nc.vector.tensor_copy(out=awT[:, ntl * 128:(ntl + 1) * 128],
                                          in_=tp3[:])
                alps = psB.tile([128, N], F32, space="PSUM", tag="b")
                nc.tensor.matmul(out=alps[:], lhsT=cem[b][:], rhs=awT[:],
                                 start=True, stop=True)
                nc.scalar.copy(out=alT[:, b * N:(b + 1) * N], in_=alps[:])

            ehr = [[keep.tile([128, 128], BF16, name=f"ehr{b}_{j}")
                    for j in range(2)] for b in range(BPC)]
            for h in range(2):
                ehps = psB.tile([128, 512], F32, space="PSUM", tag="b")
                nc.tensor.matmul(out=ehps[:], lhsT=wpk[:, 2, :],
                                 rhs=embT[:, h * 512:(h + 1) * 512],
                                 start=True, stop=False)
                nc.tensor.matmul(out=ehps[:], lhsT=wpk[:, 3, :],
                                 rhs=alT[:, h * 512:(h + 1) * 512],
                                 start=False, stop=True)
                ehT = work.tile([128, 512], F32, name="ehT")
                nc.scalar.activation(out=ehT[:], in_=ehps[:],
                                     func=mybir.ActivationFunctionType.Relu,
                                     bias=m2b)
                for j in range(4):
                    b = (h * 512 + j * 128) // N
                    jj = ((h * 512 + j * 128) % N) // 128
                    tp4 = psA.tile([128, 128], F32, space="PSUM", tag="a")
                    nc.tensor.transpose(out=tp4[:],
                                        in_=ehT[:, j * 128:(j + 1) * 128],
                                        identity=ident[:])
                    nc.vector.tensor_copy(out=ehr[b][jj][:], in_=tp4[:])

        # ================= phase C: KB graph ============================
        kbiT = [keep.tile([128, M], BF16, name=f"kbiT{b}") for b in range(BPC)]

        with tc.tile_pool(name="psS", bufs=2, space="PSUM") as psS, \
             tc.tile_pool(name="psV", bufs=2, space="PSUM") as psV, \
             tc.tile_pool(name="psT", bufs=2, space="PSUM") as psT, \
             tc.tile_pool(name="psH", bufs=2, space="PSUM") as psH:

            # original-order kb_init via onehot matmul (mask folded in)
            for b in range(BPC):
                kps = psV.tile([128, M], F32, space="PSUM", tag="v")
                for kt in range(2):
                    nc.tensor.matmul(out=kps[:], lhsT=ehr[b][kt][:],
                                     rhs=oho_sb[b][:, kt, :],
                                     start=(kt == 0), stop=(kt == 1))
                nc.scalar.copy(out=kbiT[b][:], in_=kps[:])

            # chunked pipeline; psum layouts stay 2D, the two PSUM->SBUF
            # copies do the (b, r, i) <-> (r, b, i) reshuffles via 4D APs
            CW = RPC * KR * BPC  # 512 psum cols per chunk
            for ch in range(NCH):
                rlo = ch * RPC
                nrel = min(RPC, R - rlo)
                oh2t = work2.tile([128, BPC, 2, RPC * KR], BF16, name="oh2t")
                nc.sync.dma_start(out=oh2t[:], in_=oh2c_d[ch])
                # build kb_init chunk in (b, r, i) layout: 2D outs
                bps = psS.tile([128, CW], F32, space="PSUM", tag="s")
                for b in range(BPC):
                    for kt in range(2):
                        nc.tensor.matmul(
                            out=bps[:, b * 128:(b + 1) * 128],
                            lhsT=ehr[b][kt][:],
                            rhs=oh2t[:, b, kt, :],
                            start=(b == 0 and kt == 0),
                            stop=(b == BPC - 1 and kt == 1))
                # copy + reshuffle (b, r, i) -> (r, b, i)
                kbc_t = work2.tile([128, CW], BF16, name="kbc_t")
                kbc_w = kbc_t[:].rearrange("p (r b i) -> p b r i",
                                           r=RPC, b=BPC, i=KR)
                bps_r = bps[:].rearrange("p (b r i) -> p b r i",
                                         b=BPC, r=RPC, i=KR)
                nc.vector.tensor_copy(out=kbc_w, in_=bps_r)
                # per-relation matvec: contiguous (r, b, i) slices
                vps = psV.tile([128, CW], F32, space="PSUM", tag="v")
                for j in range(nrel):
                    nc.tensor.matmul(
                        out=vps[:, j * BPC * KR:(j + 1) * BPC * KR],
                        lhsT=wt_sb[:, rlo + j, :],
                        rhs=kbc_t[:, j * BPC * KR:(j + 1) * BPC * KR],
                        start=(j == 0), stop=(j == nrel - 1))
                # copy + reshuffle back (r, b, i) -> (b, r, i); only the
                # nrel*BPC*KR cols actually written (last chunk is partial)
                nw = nrel * KR
                ksc = work2.tile([128, CW], BF16, name="ksc")
                ksc_w = ksc[:].rearrange("p (b r i) -> p b r i",
                                         b=BPC, r=RPC, i=KR)[:, :, :nrel, :]
                vps_r = vps[:, :nrel * BPC * KR].rearrange(
                    "p (r b i) -> p b r i", r=nrel, b=BPC, i=KR)
                nc.scalar.copy(out=ksc_w, in_=vps_r)
                # per-batch transposes -> sorted kb_state rows, one store
                kso = work2.tile([128, BPC, 128], BF16, name="kso")
                for b in range(BPC):
                    tps = psT.tile([128, 128], BF16, space="PSUM", tag="t")
                    nc.tensor.transpose(out=tps[:nw, :],
                                        in_=ksc[:, b * 128:b * 128 + nw],
                                        identity=identb[:])
                    nc.vector.tensor_copy(out=kso[:nw, b, :], in_=tps[:nw, :])
                base = ch * 512
                nc.sync.dma_start(
                    out=ks_sort[base:base + nw * BPC, :].rearrange(
                        "(p b) d -> p b d", b=BPC),
                    in_=kso[:nw, :, :])

            for b in range(BPC):
                nc.sync.dma_start(out=nei_sb[b][:], in_=nei_t[b])

            # aggregation (row-oriented): out rows = relu(
            #   kbi_T-slice.T @ W0_w.T + sum_kt nei_T-slice.T @ ks_rows[kt])
            for b in range(BPC):
                ksb = [small.tile([128, 128], BF16, name="ksb", bufs=8)
                       for _ in range(4)]
                for kt in range(4):
                    nc.gpsimd.indirect_dma_start(
                        out=ksb[kt][:], out_offset=None, in_=ks_sort[:],
                        in_offset=bass.IndirectOffsetOnAxis(
                            ap=vidx[:, 4 * b + kt:4 * b + kt + 1], axis=0))
                for mt in range(4):
                    hps = psH.tile([128, 128], F32, space="PSUM", tag="h")
                    nc.tensor.matmul(
                        out=hps[:], lhsT=kbiT[b][:, mt * 128:(mt + 1) * 128],
                        rhs=w0b[:], start=True, stop=False)
                    for kt in range(4):
                        nc.tensor.matmul(
                            out=hps[:],
                            lhsT=nei_sb[b][:, kt * M + mt * 128:
                                           kt * M + (mt + 1) * 128],
                            rhs=ksb[kt][:], start=False, stop=(kt == 3))
                    orow = small.tile([128, 128], F32, name="orow")
                    nc.scalar.activation(
                        out=orow[:], in_=hps[:],
                        func=mybir.ActivationFunctionType.Relu)
                    nc.sync.dma_start(
                        out=out_kb[b * M + mt * 128:b * M + (mt + 1) * 128, :],
                        in_=orow[:])

    nc.compile()
    return nc


_CACHE = {}


def _get_program():
    if "nc" not in _CACHE:
        _CACHE["nc"] = _build_program()
    return _CACHE["nc"]


def kernel(**inputs):
    in_maps = _host_prep(inputs)
    nc = _get_program()
    res = run_bass_kernel_spmd(nc, in_maps, list(range(NCORES)))
    out_ctx = np.concatenate(
        [res.results[c]["out_ctx"].reshape(BPC, EC, 128) for c in range(NCORES)])
    out_kb = np.concatenate(
        [res.results[c]["out_kb"].reshape(BPC, M, 128) for c in range(NCORES)])
    return out_ctx, out_kb


# revision 34
# speedup vs baseline: 1.1072x; 1.0753x over previous
"""Trainium2 Bass kernel for nn_EntityEncoder (gnn_message_passing).

Contract: kernel(**inputs) takes the FULL unsharded inputs (numpy) and
returns the full outputs (context_entity_hidden [32,48,128],
kb_entity_hidden [32,512,128]) as a tuple, matching reference().

Strategy: data-parallel over the batch dim (4 batches per NeuronCore,
8 cores, one SPMD program). Gathers run on-device (indirect DMA /
onehot matmuls); the per-edge relation matvec uses a relation-sorted
32-wide slot layout with a per-core slot->weight table so the
instruction stream is identical on every core. kb_state rows are
stored sorted and un-sorted by an inverse-permutation gather feeding a
row-oriented PSUM-accumulated aggregation.
"""
import sys

sys.path.insert(0, "/opt/trn_rl_repo")

from contextlib import ExitStack

import numpy as np
import ml_dtypes

import concourse.bass as bass
import concourse.tile as tile
from concourse import bacc, mybir
from concourse.bass_utils import run_bass_kernel_spmd
from concourse.masks import make_identity

# problem shapes (hardcoded per spec)
B, L, EC, N, M, D, R, V = 32, 128, 48, 256, 512, 128, 100, 40000
NCORES = 8
BPC = B // NCORES          # batches per core = 4
SLOT = 32                  # edges per matvec slot (PE col-group width)
EDG = BPC * M              # edges per core = 2048
NE = BPC * N               # entities per core = 1024
KR = 16                    # slots per (batch, relation)
RPC = 8                    # relations per 512-col PSUM chunk
NCH = (R + RPC - 1) // RPC # chunks = 13
F32 = mybir.dt.float32
BF16 = mybir.dt.bfloat16
I32 = mybir.dt.int32
NPBF = ml_dtypes.bfloat16


# ---------------------------------------------------------------- host prep

def _host_prep(inputs):
    ce_emb = np.asarray(inputs["context_emb"], np.float32)
    ce_out = np.asarray(inputs["context_outputs"], np.float32)
    cmask = np.asarray(inputs["context_mask"], np.int32)
    cpos = np.asarray(inputs["context_entity_pos"], np.int32)
    cemask = np.asarray(inputs["context_entity_mask"], np.int32)
    entity = np.asarray(inputs["entity"], np.int32)
    kbe = np.asarray(inputs["kb_entity"], np.int32)
    kbm = np.asarray(inputs["kb_entity_mask"], np.int32)
    kbc = np.asarray(inputs["kb_entity_col"], np.int32)
    nei = np.asarray(inputs["kb_entity_nei"], np.int32)
    embed_table = np.asarray(inputs["embed_table"], np.float32)
    mlp1_w = np.asarray(inputs["mlp1_w"], np.float32)
    mlp1_b = np.asarray(inputs["mlp1_b"], np.float32)
    mlp2_w = np.asarray(inputs["mlp2_w"], np.float32)
    mlp2_b = np.asarray(inputs["mlp2_b"], np.float32)
    attn_wq = np.asarray(inputs["attn_wq"], np.float32)
    attn_bq = np.asarray(inputs["attn_bq"], np.float32)
    W = np.asarray(inputs["W"], np.float32)
    W0_w = np.asarray(inputs["W0_w"], np.float32)

    # fixed (relation, batch, slot16) grid: KR slots per (b, r); NCH chunks
    # of RPC=8 relations; all layout decisions static across cores.
    assert max(np.bincount(kbc[b], minlength=R).max() for b in range(B)) <= KR

    m1 = mlp1_w.T.reshape(2, 128, 128).transpose(1, 0, 2)
    m2 = mlp2_w.T.reshape(2, 128, 128).transpose(1, 0, 2)
    wpack = np.stack([m1[:, 0], m1[:, 1], m2[:, 0], m2[:, 1],
                      attn_wq.T, W0_w.T], axis=1)
    bpack = np.stack([mlp1_b, mlp2_b, attn_bq], axis=1)
    shared = dict(
        wpack=np.ascontiguousarray(wpack.astype(np.float32)),
        bpack=np.ascontiguousarray(bpack.astype(np.float32)),
        wtp=np.ascontiguousarray(W.transpose(2, 0, 1).astype(NPBF)),
        w0b=np.ascontiguousarray(W0_w.T.astype(NPBF)),
        emb_tbl=embed_table,
    )

    in_maps = []
    for c in range(NCORES):
        sl = slice(BPC * c, BPC * (c + 1))
        amask = np.ascontiguousarray(
            np.where(cmask[sl] > 0, 0.0, -1e9).astype(np.float32).reshape(
                1, BPC * 128))
        oh1 = np.zeros((BPC, L, EC), np.float32)
        for b in range(BPC):
            oh1[b, cpos[sl][b], np.arange(EC)] = cemask[sl][b].astype(np.float32)

        kbe_c, kbm_c = kbe[sl], kbm[sl]
        kbc_c = kbc[sl]
        # oho2: per-batch onehot into the (r, slot16) grid (no mask: the kb
        # mask is folded into nei, and pads never get read back)
        oh2 = np.zeros((BPC, N, R * KR), np.float32)
        inv = np.zeros(EDG, np.int32)
        for b in range(BPC):
            order = np.lexsort((np.arange(M), kbc_c[b]))
            i_in_r = np.zeros(R, np.int32)
            for m_ in order:
                r = kbc_c[b, m_]
                pos = r * KR + i_in_r[r]
                i_in_r[r] += 1
                oh2[b, kbe_c[b, m_], pos] = 1.0
                ch_, pc_ = pos // 128, pos % 128
                inv[b * M + m_] = ch_ * 512 + pc_ * 4 + b
        # chunked upload [NCH, 128, BPC, 2, RPC*KR] (zero-padded past R*KR)
        oh2p = np.zeros((BPC, N, NCH * RPC * KR), np.float32)
        oh2p[:, :, :R * KR] = oh2
        oh2c = np.ascontiguousarray(
            oh2p.reshape(BPC, 2, 128, NCH, RPC * KR)
                .transpose(3, 2, 0, 1, 4))  # [NCH, 128, BPC, 2, 128]

        ipack = np.ascontiguousarray(np.concatenate([
            entity[sl].ravel().reshape(NE // 128, 128).T.astype(np.int32),
            inv.reshape(EDG // 128, 128).T], axis=1))

        # original-order kb_init onehot (mask folded in) for the W0 term
        oho = np.zeros((BPC, N, M), np.float32)
        for b in range(BPC):
            oho[b, kbe_c[b], np.arange(M)] = kbm_c[b].astype(np.float32)
        oh_orig = np.ascontiguousarray(
            oho.reshape(BPC, 2, 128, M).transpose(0, 2, 1, 3).astype(NPBF))

        # degree-normalized transposed neighbor matrix (mask folded in)
        nei_c = nei[sl].astype(np.float32)
        deg = np.clip(nei_c.sum(axis=2), 1.0, None)
        nnT = (nei_c / deg[:, :, None]).transpose(0, 2, 1)
        nnT = nnT * kbm_c.astype(np.float32)[:, :, None]
        nei_t = np.ascontiguousarray(
            nnT.reshape(BPC, 4, 128, M).transpose(0, 2, 1, 3).reshape(
                BPC, 128, 4 * M).astype(NPBF))

        m = dict(shared)
        m.update(
            ce_emb=np.ascontiguousarray(ce_emb[sl]),
            ce_out=np.ascontiguousarray(ce_out[sl]),
            amask=amask,
            onehot1=np.ascontiguousarray(oh1.transpose(1, 0, 2)),
            ipack=ipack,
            oh2c=oh2c.astype(NPBF),
            oh_orig=oh_orig,
            nei_t=nei_t,
        )
        in_maps.append(m)
    return in_maps


# ------------------------------------------------------------- bass program

def _build_program():
    nto = EDG // 128
    nc = bacc.Bacc("TRN2", target_bir_lowering=False, debug=False,
                   num_devices=NCORES)

    def din(name, shape, dt=F32):
        return nc.dram_tensor(name, list(shape), dt, kind="ExternalInput").ap()

    ce_emb = din("ce_emb", (BPC, 128, 128))
    ce_out = din("ce_out", (BPC, 128, 128))
    amask = din("amask", (1, BPC * 128))
    onehot1 = din("onehot1", (128, BPC, EC))
    wpack = din("wpack", (128, 6, 128))
    bpack = din("bpack", (128, 3))
    wtp = din("wtp", (128, R, 128), BF16)
    w0b_d = din("w0b", (128, 128), BF16)
    ipack = din("ipack", (128, NE // 128 + nto), I32)
    emb_tbl = din("emb_tbl", (V, 128))
    oh2c_d = din("oh2c", (NCH, 128, BPC, 2, RPC * KR), BF16)
    oh_orig = din("oh_orig", (BPC, 128, 2, M), BF16)
    nei_t = din("nei_t", (BPC, 128, 4 * M), BF16)

    out_ctx = nc.dram_tensor("out_ctx", [BPC * EC, 128], F32,
                             kind="ExternalOutput").ap()
    out_kb = nc.dram_tensor("out_kb", [EDG, 128], F32,
                            kind="ExternalOutput").ap()

    ks_sort = nc.dram_tensor("ks_sort", [(NCH - 1) * 512 + 256, 128], BF16).ap()

    with tile.TileContext(nc) as tc, ExitStack() as ctx:
        consts = ctx.enter_context(tc.tile_pool(name="consts", bufs=1))
        big = ctx.enter_context(tc.tile_pool(name="big", bufs=1))
        work = ctx.enter_context(tc.tile_pool(name="work", bufs=3))
        work2 = ctx.enter_context(tc.tile_pool(name="work2", bufs=3))
        keep = ctx.enter_context(tc.tile_pool(name="keep", bufs=1))
        small = ctx.enter_context(tc.tile_pool(name="small", bufs=4))

        # ---- sync queue: small latency-critical loads, indices first
        def ld(pool, shape, src, dt=F32, name=None):
            t = pool.tile(shape, dt, name=name)
            nc.sync.dma_start(out=t[:], in_=src)
            return t

        ipk = ld(consts, [128, NE // 128 + nto], ipack[:], I32, "ipk")
        eidx = ipk[:, 0:NE // 128]
        vidx = ipk[:, NE // 128:]
        oh1 = ld(consts, [128, BPC, EC], onehot1[:], name="oh1")
        cem = [ld(consts, [128, 128], ce_emb[b], name=f"cem{b}")
               for b in range(BPC)]
        ceo = [ld(consts, [128, 128], ce_out[b], name=f"ceo{b}")
               for b in range(BPC)]
        wpk = ld(consts, [128, 6, 128], wpack[:], name="wpk")
        bpk = ld(consts, [128, 3], bpack[:], name="bpk")
        amr = ld(consts, [1, BPC * 128], amask[:], name="amr")
        w0b = ld(consts, [128, 128], w0b_d[:], BF16, "w0b")

        # ---- gpsimd: embedding gathers early (phase B input)
        embr = [[keep.tile([128, 128], F32, name=f"embr{b}_{j}")
                 for j in range(2)] for b in range(BPC)]
        for b in range(BPC):
            for j in range(2):
                nc.gpsimd.indirect_dma_start(
                    out=embr[b][j][:], out_offset=None, in_=emb_tbl[:],
                    in_offset=bass.IndirectOffsetOnAxis(
                        ap=eidx[:, 2 * b + j:2 * b + j + 1], axis=0))

        # identity built after the gathers are issued (gpsimd FIFO order);
        # first transpose consumer runs well after
        ident = consts.tile([128, 128], F32)
        make_identity(nc, ident[:])
        ones1 = consts.tile([1, 128], F32)
        nc.gpsimd.memset(ones1[:], 1.0)
        identb = consts.tile([128, 128], BF16)
        nc.vector.tensor_copy(out=identb[:], in_=ident[:])

        # ---- bulk background loads on sync HWDGE rings
        oho_sb = [big.tile([128, 2, M], BF16, name=f"oho{b}") for b in range(BPC)]
        for b in range(BPC):
            nc.sync.dma_start(out=oho_sb[b][:], in_=oh_orig[b])
        wt_sb = big.tile([128, R, 128], BF16)
        for q in range(4):
            qs = R // 4
            nc.sync.dma_start(out=wt_sb[:, q * qs:(q + 1) * qs, :],
                              in_=wtp[:, q * qs:(q + 1) * qs, :])
        nei_sb = [big.tile([128, 4 * M], BF16, name=f"nei{b}") for b in range(BPC)]

        m1b, m2b, bqs = bpk[:, 0:1], bpk[:, 1:2], bpk[:, 2:3]

        with tc.tile_pool(name="psA", bufs=2, space="PSUM") as psA, \
             tc.tile_pool(name="psB", bufs=5, space="PSUM") as psB:

            # ================= phase A: context-entity hidden ================
            cehT = [keep.tile([128, BPC * EC], F32, name=f"cehT{k}")
                    for k in range(2)]
            for b in range(BPC):
                for k, src in ((0, cem[b]), (1, ceo[b])):
                    aps = psA.tile([128, EC], F32, space="PSUM", tag="a")
                    nc.tensor.matmul(out=aps[:], lhsT=src[:],
                                     rhs=oh1[:, b, :], start=True, stop=True)
                    nc.scalar.copy(out=cehT[k][:, b * EC:(b + 1) * EC],
                                   in_=aps[:])
            o1ps = psB.tile([128, BPC * EC], F32, space="PSUM", tag="b")
            nc.tensor.matmul(out=o1ps[:], lhsT=wpk[:, 0, :], rhs=cehT[0][:],
                             start=True, stop=False)
            nc.tensor.matmul(out=o1ps[:], lhsT=wpk[:, 1, :], rhs=cehT[1][:],
                             start=False, stop=True)
            o1T = work.tile([128, BPC * EC], F32)
            nc.scalar.activation(out=o1T[:], in_=o1ps[:],
                                 func=mybir.ActivationFunctionType.Relu,
                                 bias=m1b)
            for h in range(2):
                tp = psA.tile([96, 128], F32, space="PSUM", tag="a")
                nc.tensor.transpose(out=tp[:], in_=o1T[:, h * 96:(h + 1) * 96],
                                    identity=ident[:])
                o1r = work.tile([96, 128], F32)
                nc.vector.tensor_copy(out=o1r[:], in_=tp[:])
                nc.sync.dma_start(out=out_ctx[h * 96:(h + 1) * 96, :],
                                  in_=o1r[:])

            # ================= phase B: entity attention + mlp2 ==============
            ceT = [keep.tile([128, 128], F32, name=f"ceT{b}")
                   for b in range(BPC)]
            embT = keep.tile([128, BPC * N], F32)
            for b in range(BPC):
                tp = psA.tile([128, 128], F32, space="PSUM", tag="a")
                nc.tensor.transpose(out=tp[:], in_=cem[b][:], identity=ident[:])
                nc.scalar.copy(out=ceT[b][:], in_=tp[:])
                for j in range(2):
                    tp2 = psA.tile([128, 128], F32, space="PSUM", tag="a")
                    nc.tensor.transpose(out=tp2[:], in_=embr[b][j][:],
                                        identity=ident[:])
                    nc.vector.tensor_copy(
                        out=embT[:, b * N + j * 128:b * N + (j + 1) * 128],
                        in_=tp2[:])

            qT = keep.tile([128, BPC * N], F32)
            for h in range(2):
                qps = psB.tile([128, 512], F32, space="PSUM", tag="b")
                nc.tensor.matmul(out=qps[:], lhsT=wpk[:, 4, :],
                                 rhs=embT[:, h * 512:(h + 1) * 512],
                                 start=True, stop=True)
                nc.scalar.activation(out=qT[:, h * 512:(h + 1) * 512],
                                     in_=qps[:],
                                     func=mybir.ActivationFunctionType.Identity,
                                     bias=bqs)

            sps = []
            for b in range(BPC):
                ps = psB.tile([128, 256], F32, space="PSUM", tag="b")
                sps.append(ps)
                for ntl in range(2):
                    nc.tensor.matmul(
                        out=ps[:, ntl * 128:(ntl + 1) * 128],
                        lhsT=qT[:, b * N + ntl * 128:b * N + (ntl + 1) * 128],
                        rhs=ceT[b][:], start=(ntl == 0), stop=False)
                for ntl in range(2):
                    nc.tensor.matmul(
                        out=ps[:, ntl * 128:(ntl + 1) * 128],
                        lhsT=ones1[:], rhs=amr[:, b * 128:(b + 1) * 128],
                        start=False, stop=(ntl == 1))

            att = []
            for b in range(BPC):
                sc = small.tile([128, 2, 128], F32, name="sc")
                nc.scalar.activation(out=sc[:], in_=sps[b][:].rearrange(
                    "p (t l) -> p t l", t=2),
                    func=mybir.ActivationFunctionType.Exp)
                ssum = small.tile([128, 2], F32, name="ssum")
                nc.vector.tensor_reduce(out=ssum[:], in_=sc[:],
                                        axis=mybir.AxisListType.X,
                                        op=mybir.AluOpType.add)
                rs = small.tile([128, 2], F32, name="rs")
                nc.vector.reciprocal(out=rs[:], in_=ssum[:])
                rsb = bass.AP(tensor=rs.tensor, offset=rs.offset,
                              ap=[rs.ap[0], [rs.ap[1][0], 2], [0, 128]])
                nc.vector.tensor_tensor(out=sc[:], in0=sc[:], in1=rsb,
                                        op=mybir.AluOpType.mult)
                att.append(sc)

            alT = keep.tile([128, BPC * N], F32)
            for b in range(BPC):
                awT = work.tile([128, N], F32)
                for ntl in range(2):
                    tp3 = psA.tile([128, 128], F32, space="PSUM", tag="a")
                    nc.tensor.transpose(out=tp3[:], in_=att[b][:, ntl, :],
                                        identity=ident[:])
                    nc.vector.tensor_copy(out=awT[:, ntl * 128:(ntl + 1) * 128],
                                          in_=tp3[:])
                alps = psB.tile([128, N], F32, space="PSUM", tag="b")
                nc.tensor.matmul(out=alps[:], lhsT=cem[b][:], rhs=awT[:],
                                 start=True, stop=True)
                nc.scalar.copy(out=alT[:, b * N:(b + 1) * N], in_=alps[:])

            ehr = [[keep.tile([128, 128], BF16, name=f"ehr{b}_{j}")
                    for j in range(2)] for b in range(BPC)]
            for h in range(2):
                ehps = psB.tile([128, 512], F32, space="PSUM", tag="b")
                nc.tensor.matmul(out=ehps[:], lhsT=wpk[:, 2, :],
                                 rhs=embT[:, h * 512:(h + 1) * 512],
                                 start=True, stop=False)
                nc.tensor.matmul(out=ehps[:], lhsT=wpk[:, 3, :],
                                 rhs=alT[:, h * 512:(h + 1) * 512],
                                 start=False, stop=True)
                ehT = work.tile([128, 512], F32, name="ehT")
                nc.scalar.activation(out=ehT[:], in_=ehps[:],
                                     func=mybir.ActivationFunctionType.Relu,
                                     bias=m2b)
                for j in range(4):
                    b = (h * 512 + j * 128) // N
                    jj = ((h * 512 + j * 128) % N) // 128
                    tp4 = psA.tile([128, 128], F32, space="PSUM", tag="a")
                    nc.tensor.transpose(out=tp4[:],
                                        in_=ehT[:, j * 128:(j + 1) * 128],
                                        identity=ident[:])
                    nc.vector.tensor_copy(out=ehr[b][jj][:], in_=tp4[:])

        # ================= phase C: KB graph ============================
        kbiT = [keep.tile([128, M], BF16, name=f"kbiT{b}") for b in range(BPC)]

        with tc.tile_pool(name="psS", bufs=2, space="PSUM") as psS, \
             tc.tile_pool(name="psV", bufs=2, space="PSUM") as psV, \
             tc.tile_pool(name="psT", bufs=2, space="PSUM") as psT, \
             tc.tile_pool(name="psH", bufs=2, space="PSUM") as psH:

            # original-order kb_init via onehot matmul (mask folded in)
            for b in range(BPC):
                kps = psV.tile([128, M], F32, space="PSUM", tag="v")
                for kt in range(2):
                    nc.tensor.matmul(out=kps[:], lhsT=ehr[b][kt][:],
                                     rhs=oho_sb[b][:, kt, :],
                                     start=(kt == 0), stop=(kt == 1))
                nc.scalar.copy(out=kbiT[b][:], in_=kps[:])

            # chunked pipeline; psum layouts stay 2D, the two PSUM->SBUF
            # copies do the (b, r, i) <-> (r, b, i) reshuffles via 4D APs
            CW = RPC * KR * BPC  # 512 psum cols per chunk
            for ch in range(NCH):
                rlo = ch * RPC
                nrel = min(RPC, R - rlo)
                oh2t = work2.tile([128, BPC, 2, RPC * KR], BF16, name="oh2t")
                nc.sync.dma_start(out=oh2t[:], in_=oh2c_d[ch])
                # build kb_init chunk in (b, r, i) layout: 2D outs
                bps = psS.tile([128, CW], F32, space="PSUM", tag="s")
                for b in range(BPC):
                    for kt in range(2):
                        nc.tensor.matmul(
                            out=bps[:, b * 128:(b + 1) * 128],
                            lhsT=ehr[b][kt][:],
                            rhs=oh2t[:, b, kt, :],
                            start=(b == 0 and kt == 0),
                            stop=(b == BPC - 1 and kt == 1))
                # copy + reshuffle (b, r, i) -> (r, b, i)
                kbc_t = work2.tile([128, CW], BF16, name="kbc_t")
                kbc_w = kbc_t[:].rearrange("p (r b i) -> p b r i",
                                           r=RPC, b=BPC, i=KR)
                bps_r = bps[:].rearrange("p (b r i) -> p b r i",
                                         b=BPC, r=RPC, i=KR)
                nc.vector.tensor_copy(out=kbc_w, in_=bps_r)
                # per-relation matvec: contiguous (r, b, i) slices
                vps = psV.tile([128, CW], F32, space="PSUM", tag="v")
                for j in range(nrel):
                    nc.tensor.matmul(
                        out=vps[:, j * BPC * KR:(j + 1) * BPC * KR],
                        lhsT=wt_sb[:, rlo + j, :],
                        rhs=kbc_t[:, j * BPC * KR:(j + 1) * BPC * KR],
                        start=(j == 0), stop=(j == nrel - 1))
                # copy + reshuffle back (r, b, i) -> (b, r, i); only the
                # nrel*BPC*KR cols actually written (last chunk is partial)
                nw = nrel * KR
                ksc = work2.tile([128, CW], BF16, name="ksc")
                ksc_w = ksc[:].rearrange("p (b r i) -> p b r i",
                                         b=BPC, r=RPC, i=KR)[:, :, :nrel, :]
                vps_r = vps[:, :nrel * BPC * KR].rearrange(
                    "p (r b i) -> p b r i", r=nrel, b=BPC, i=KR)
                nc.scalar.copy(out=ksc_w, in_=vps_r)
                # per-batch transposes -> sorted kb_state rows, one store
                kso = work2.tile([128, BPC, 128], BF16, name="kso")
                for b in range(BPC):
                    tps = psT.tile([128, 128], BF16, space="PSUM", tag="t")
                    nc.tensor.transpose(out=tps[:nw, :],
                                        in_=ksc[:, b * 128:b * 128 + nw],
                                        identity=identb[:])
                    nc.vector.tensor_copy(out=kso[:nw, b, :], in_=tps[:nw, :])
                base = ch * 512
                nc.sync.dma_start(
                    out=ks_sort[base:base + nw * BPC, :].rearrange(
                        "(p b) d -> p b d", b=BPC),
                    in_=kso[:nw, :, :])

            for b in range(BPC):
                nc.sync.dma_start(out=nei_sb[b][:], in_=nei_t[b])

            # aggregation (row-oriented): out rows = relu(
            #   kbi_T-slice.T @ W0_w.T + sum_kt nei_T-slice.T @ ks_rows[kt])
            for b in range(BPC):
                ksb = [small.tile([128, 128], BF16, name="ksb", bufs=8)
                       for _ in range(4)]
                for kt in range(4):
                    nc.gpsimd.indirect_dma_start(
                        out=ksb[kt][:], out_offset=None, in_=ks_sort[:],
                        in_offset=bass.IndirectOffsetOnAxis(
                            ap=vidx[:, 4 * b + kt:4 * b + kt + 1], axis=0))
                for mt in range(4):
                    hps = psH.tile([128, 128], F32, space="PSUM", tag="h")
                    nc.tensor.matmul(
                        out=hps[:], lhsT=kbiT[b][:, mt * 128:(mt + 1) * 128],
                        rhs=w0b[:], start=True, stop=False)
                    for kt in range(4):
                        nc.tensor.matmul(
                            out=hps[:],
                            lhsT=nei_sb[b][:, kt * M + mt * 128:
                                           kt * M + (mt + 1) * 128],
                            rhs=ksb[kt][:], start=False, stop=(kt == 3))
                    orow = small.tile([128, 128], F32, name="orow")
                    nc.scalar.activation(
                        out=orow[:], in_=hps[:],
                        func=mybir.ActivationFunctionType.Relu)
                    nc.sync.dma_start(
                        out=out_kb[b * M + mt * 128:b * M + (mt + 1) * 128, :],
                        in_=orow[:])

    nc.compile()
    return nc


_CACHE = {}


def _get_program():
    if "nc" not in _CACHE:
        _CACHE["nc"] = _build_program()
    return _CACHE["nc"]


def kernel(**inputs):
    in_maps = _host_prep(inputs)
    nc = _get_program()
    res = run_bass_kernel_spmd(nc, in_maps, list(range(NCORES)))
    out_ctx = np.concatenate(
        [res.results[c]["out_ctx"].reshape(BPC, EC, 128) for c in range(NCORES)])
    out_kb = np.concatenate(
        [res.results[c]["out_kb"].reshape(BPC, M, 128) for c in range(NCORES)])
    return out_ctx, out_kb
